# revision 87
# baseline (speedup 1.0000x reference)
"""MQA self-attention kernel for Trainium2, 8 NeuronCores.

Reference computation (fp32):
    q = x @ wq.T + bq        -> [B,S,1024] -> heads via (hidden num_heads) split
    k = x @ wk.T + bk        -> [B,S,64]  (single shared KV head)
    v = x @ wv.T + bv
    scores = q @ k.T / 8 ; attn = softmax(scores) ; h = attn @ v
    out = merge_heads(h) @ wo.T + bo

Sharding (8 cores, no collectives): core c handles batch b=c//4 and head
group g=c%4 (4 of the 16 q-heads).  The shared K/V head is replicated.
Each core returns the partial output h_g @ wo_g.T [S, D]; the host sums
the 4 head-group partials per batch and adds the bias terms.

Math notes:
 - bk provably cancels in softmax; bv is folded into the host-side output
   bias (softmax rows sum to 1); softmax runs without max subtraction
   (scores ~ N(0,1), exp stays within bf16/f32 range).

Device pipeline (all operands bf16, PSUM f32; the Activation engine's
exp throughput ~133us/core is the hard floor, so every other engine is
kept strictly below it):
 - xT/weights land as bf16 (halves the input DMA); issue queues are
   split SP/ACT because each DMA costs ~1.2-2us of serialized issue.
 - projections produce QT [2 heads stacked per 128 partitions] and KT
   (two partition-offset copies so odd/even heads get offset-matched
   operands); V' = [V|1] is computed DIRECTLY in [keys, dims] layout
   (lhsT = xT chunk, rhs = wvT) so no transposes or eviction chains
   sit in front of the early scores.
 - scores_T[k, q] per head per 1024-query block; exp on ScalarE in
   [128,1024] blocks writing bf16.
 - PV runs dense: h_un[q, 65] += exp_T[:, qtile].T @ V' accumulated over
   key tiles in PSUM (2x fewer PE rows than the hT-layout alternative).
   PSUM accumulation groups are per-2KB-bank: one start/stop per bank.
 - normalize on eviction via per-partition reciprocal multiply; pairs of
   heads share an SBUF buffer flipped to hT layout by one DMA-transpose
   per 512-col group (PE-transpose for the final pair, lower latency).
 - half-1 projections, V' chains and out-projection chunks are deferred
   PE tasks popped one per exp slot; emission order encodes the RAW/WAR
   dependencies while tile_wait_until schedule floors stop the greedy
   scheduler from hoisting them in front of the critical exp chain.
 - the final block's out-projection flushes after the scores/h_un banks
   are reclaimed, with a 4-deep PSUM pipeline, 2:1 DVE:ACT evictions
   and 2-chunk batched output DMAs.
"""

from collections import deque

import numpy as np

NUM_HEADS = 16
Dh = 64
B, S, D = 2, 2048, 1024
G = 4            # head groups (cores per batch)
HG = 4           # heads per group
QD = HG * Dh     # 256 local q dims
NK = D // 128    # 8 contraction tiles for projections
NSK = S // 128   # 16 key tiles
W = 512          # matmul moving width
HALF = 1024      # query block / projection column half
N_CORES = 8

_CACHE = {}
_DEBUG = False


def _build_nc():
    from contextlib import ExitStack

    import concourse.bass as bass
    import concourse.mybir as mybir
    import concourse.tile as tile
    from concourse import bacc
    from concourse.masks import make_identity

    F32 = mybir.dt.float32
    BF16 = mybir.dt.bfloat16
    EXP = mybir.ActivationFunctionType.Exp

    nc = bacc.Bacc("TRN2", target_bir_lowering=False, debug=False)

    xT = nc.declare_dram_parameter("xT", [D, S], BF16, isOutput=False)
    wqT = nc.declare_dram_parameter("wqT", [D, QD], BF16, isOutput=False)
    wkvT = nc.declare_dram_parameter("wkvT", [D, 128], BF16, isOutput=False)
    woT = nc.declare_dram_parameter("woT", [QD, D], BF16, isOutput=False)
    bqp = nc.declare_dram_parameter("bq", [QD, 1], F32, isOutput=False)
    part = nc.declare_dram_parameter("part", [S, D], BF16, isOutput=True)
    if _DEBUG:
        dbg = {
            "d_qt": nc.declare_dram_parameter("d_qt", [128, 2 * S], BF16, isOutput=True),
            "d_kt2": nc.declare_dram_parameter("d_kt2", [128, S], BF16, isOutput=True),
            "d_v1": nc.declare_dram_parameter("d_v1", [128, NSK * 65], BF16, isOutput=True),
            "d_ht": nc.declare_dram_parameter("d_ht", [128, 2 * S], BF16, isOutput=True),
        }

    with tile.TileContext(nc) as tc, ExitStack() as ctx:
        const = ctx.enter_context(tc.tile_pool(name="const", bufs=1))
        persist = ctx.enter_context(tc.tile_pool(name="persist", bufs=1))

        wq_sb = const.tile([128, NK * QD], BF16)    # ktile kt at cols [kt*QD:+QD]
        wkv_sb = const.tile([128, NK * 128], BF16)  # cols 0:64 = wkT, 64:128 = wvT
        wo_sb = const.tile([128, 2 * D], BF16)      # q-ktile p at cols [p*D:+D]
        bq_sb = const.tile([128, 2], F32)
        ident = const.tile([128, 128], BF16)        # for the tail PE transposes

        qt_sb = persist.tile([128, 2 * S], BF16)    # pt p cols [p*S:+S]; rows 0:64 head 2p, 64:128 head 2p+1
        kt2_sb = persist.tile([128, S], BF16)       # KT duplicated rows 0:64 and 64:128
        v1_sb = persist.tile([128, NSK * 65], BF16)  # V' tile sk at cols [sk*65:+65]
        ht_sb = persist.tile([128, 2 * S], BF16)    # hT, q-ktile p at cols [p*S:+S]

        make_identity(nc, ident[:])
        nc.vector.memset(v1_sb[:], 1.0)  # pre-fill the softmax-denominator columns

        # ---- DMAs + half-0 projections --------------------------------
        # x lands in 4 tiles of 2 ktiles (half 0, fine overlap with the
        # projection chains) + 2 tiles of 4 ktiles (half 1).  Issue queues
        # are spread over SP/ACT/DVE so the ~1.2us per-DMA issue cost
        # doesn't serialize the critical path.
        xp = ctx.enter_context(tc.tile_pool(name="xp", bufs=1))
        x0 = [xp.tile([128, 2 * HALF], BF16, name=f"x0_{j}") for j in range(4)]
        x1 = [xp.tile([128, 2 * HALF], BF16, name=f"x1_{j}") for j in range(4)]

        def dma_x(eng, tiles, j, hf):
            eng.dma_start(
                tiles[j][:].rearrange("p (k c) -> p k c", c=HALF),
                xT[j * 256:(j + 1) * 256, hf * HALF:(hf + 1) * HALF].rearrange(
                    "(k p) c -> p k c", p=128),
            )

        # weights issue from the ACT queue (idle until the first exp at
        # ~12us), x tiles from SP: neither queue's serialized ~1.2-2us
        # per-DMA issue cost then gates the critical path
        nc.scalar.dma_start(
            wkv_sb[:].rearrange("p (k c) -> p k c", c=128),
            wkvT[:, :].rearrange("(k p) c -> p k c", p=128),
        )
        nc.scalar.dma_start(
            wq_sb[:].rearrange("p (k c) -> p k c", c=QD),
            wqT[:, :].rearrange("(k p) c -> p k c", p=128),
        )
        for j in range(4):
            dma_x(nc.sync, x0, j, 0)
        for j in range(4):
            dma_x(nc.sync, x1, j, 1)
        for p in range(2):
            nc.scalar.dma_start(bq_sb[:, p:p + 1], bqp[p * 128:(p + 1) * 128, :])
        nc.scalar.dma_start(
            wo_sb[:].rearrange("p (a c) -> p a c", c=D),
            woT[:, :].rearrange("(a p) c -> p a c", p=128),
        )

        def xcol(kt, hf, n):
            tiles = x0 if hf == 0 else x1
            return tiles[kt // 2][:, (kt % 2) * HALF + n * W:(kt % 2) * HALF + (n + 1) * W]

        def proj_kt_step(ps, wsb_col, kt, hf, width):
            for n in range(HALF // W):
                nc.tensor.matmul(
                    ps[:, n * W:(n + 1) * W],
                    lhsT=wsb_col(kt),
                    rhs=xcol(kt, hf, n),
                    start=(kt == 0), stop=(kt == NK - 1),
                )

        wkv_col = lambda kt: wkv_sb[:, kt * 128:(kt + 1) * 128]
        wq0_col = lambda kt: wq_sb[:, kt * QD:kt * QD + 128]
        wq1_col = lambda kt: wq_sb[:, kt * QD + 128:kt * QD + 256]

        with tc.tile_pool(name="p0", bufs=1, space="PSUM") as p0:
            vk_ps = p0.tile([128, HALF], F32, name="vk")
            q0_ps = p0.tile([128, HALF], F32, name="q0")
            # vk + q0 gate the first exp; everything else is deferred with
            # schedule-time floors so it can't precede the early scores
            for j in range(4):
                # vk first within each freshly-landed x tile: the K chain
                # (which gates the first scores) finishes ~0.8us earlier
                proj_kt_step(vk_ps, wkv_col, 2 * j, 0, HALF)
                proj_kt_step(vk_ps, wkv_col, 2 * j + 1, 0, HALF)
                proj_kt_step(q0_ps, wq0_col, 2 * j, 0, HALF)
                proj_kt_step(q0_ps, wq0_col, 2 * j + 1, 0, HALF)
            # halved evictions: the first scores matmul only needs the first
            # key tile and query half, so don't gate it on full-width copies
            nc.scalar.copy(kt2_sb[0:64, 0:W], vk_ps[0:64, 0:W])
            nc.vector.tensor_scalar_add(qt_sb[:, 0:W], q0_ps[:, 0:W], bq_sb[:, 0:1])
            nc.scalar.copy(kt2_sb[0:64, W:HALF], vk_ps[0:64, W:HALF])
            nc.vector.tensor_scalar_add(qt_sb[:, W:HALF], q0_ps[:, W:HALF],
                                        bq_sb[:, 0:1])
            nc.gpsimd.tensor_copy(kt2_sb[64:128, 0:HALF], kt2_sb[0:64, 0:HALF])

        # ---- Phase 2: attention, with deferred fill-in PE tasks -------
        expp = ctx.enter_context(tc.tile_pool(name="expp", bufs=14))
        hpp = ctx.enter_context(tc.tile_pool(name="hpp", bufs=2))
        smalls = ctx.enter_context(tc.tile_pool(name="smalls", bufs=4))
        osbp = ctx.enter_context(tc.tile_pool(name="osbp", bufs=3))
        # scores/h_un PSUM lives in its own stack so the final-block flush
        # can reclaim those banks for a deeper out-projection pipeline.
        # hups is created first: its banks land on the just-freed phase-1
        # chain banks (whose WAR hold clears before the first PV), keeping
        # the second scores buffer's banks free of phase-1 dependencies.
        scph_stack = ExitStack()
        hups = scph_stack.enter_context(tc.tile_pool(name="hups", bufs=1, space="PSUM"))
        scps = scph_stack.enter_context(tc.tile_pool(name="scps", bufs=2, space="PSUM"))

        state = {}

        def mk_vk1(kts):
            def t():
                if "vk1" not in state:
                    pvk = pvk_stack.enter_context(
                        tc.tile_pool(name="pvk", bufs=1, space="PSUM", side="right"))
                    state["vk1"] = pvk.tile([128, HALF], F32, name="vkps")
                for kt in kts:
                    proj_kt_step(state["vk1"], wkv_col, kt, 1, HALF)
            return t

        def t_kv1_evict_a():
            # first half of the K eviction unblocks scores sk8-11 sooner
            nc.vector.tensor_copy(kt2_sb[0:64, HALF:HALF + W],
                                  state["vk1"][0:64, 0:W])

        def t_kv1_evict_b():
            vk1 = state.pop("vk1")
            nc.vector.tensor_copy(kt2_sb[0:64, HALF + W:S], vk1[0:64, W:HALF])
            nc.gpsimd.tensor_copy(kt2_sb[64:128, HALF:S], kt2_sb[0:64, HALF:S])
            pvk_stack.close()

        def mk_vdirect(era, sks, close=False):
            # V in [keys, dims] layout computed directly (lhsT = xT chunk,
            # rhs = wvT): no PE transposes, no eviction dependency chains —
            # each key tile is its own 8-matmul chain plus one DVE eviction
            def t():
                if era not in state:
                    state[era] = ExitStack()
                    state[era + "_p"] = state[era].enter_context(
                        tc.tile_pool(name=era, bufs=2, space="PSUM", side="right"))
                for sk in sks:
                    vd = state[era + "_p"].tile([128, Dh], F32, name="vd")
                    hf, c0 = (0, sk * 128) if sk < NSK // 2 else (1, sk * 128 - HALF)
                    tiles = x0 if hf == 0 else x1
                    for kt in range(NK):
                        nc.tensor.matmul(
                            vd[:],
                            lhsT=tiles[kt // 2][:, (kt % 2) * HALF + c0:
                                                (kt % 2) * HALF + c0 + 128],
                            rhs=wkv_sb[:, kt * 128 + 64:kt * 128 + 128],
                            start=(kt == 0), stop=(kt == NK - 1),
                        )
                    nc.vector.tensor_copy(v1_sb[:, sk * 65:sk * 65 + 64], vd[:])
                if close:
                    state.pop(era).close()
                    state.pop(era + "_p")
            return t

        def mk_q1(which, col_fn, kts, hf):
            def t():
                if which in state:
                    ps = state[which]
                else:
                    if "pq" not in state:
                        state["pq"] = pq_stack.enter_context(
                            tc.tile_pool(name="pq", bufs=1, space="PSUM", side="right"))
                    ps = state[which] = state["pq"].tile([128, HALF], F32, name="qps")
                for kt in kts:
                    proj_kt_step(ps, col_fn, kt, hf, HALF)
            return t

        def mk_q1_evict(which, pt, hf):
            def t():
                ps = state.pop(which)
                c0 = pt * S + hf * HALF
                nc.vector.tensor_scalar_add(
                    qt_sb[:, c0:c0 + HALF], ps[:], bq_sb[:, pt:pt + 1]
                )
            return t

        def t_close_p1b():
            pq_stack.close()

        # (floor_us, task): the floor is a schedule-sim wait that keeps this
        # fill-in work from being greedily ordered ahead of the exp chain
        tasks = deque(
            [(12.6, mk_vk1(range(0, 2))), (14.1, mk_vk1(range(2, 4))),
             (15.6, mk_vk1(range(4, 6))), (17.1, mk_vk1(range(6, 8))),
             (17.7, t_kv1_evict_a), (18.1, t_kv1_evict_b),
             (17.2, mk_vdirect("vd", (0, 1))),
             (18.0, mk_vdirect("vd", (2, 3))),
             (18.8, mk_vdirect("vd", (4, 5))),
             (19.6, mk_vdirect("vd", (6, 7))),
             (20.4, mk_vdirect("vd", (8, 9))),
             (21.2, mk_vdirect("vd", (10, 11))),
             (22.0, mk_vdirect("vd", (12, 13))),
             (22.8, mk_vdirect("vd", (14, 15), close=True)),
             (24.4, mk_q1("q1h0", wq1_col, range(0, 2), 0)),
             (25.2, mk_q1("q1h0", wq1_col, range(2, 4), 0)),
             (26.0, mk_q1("q1h0", wq1_col, range(4, 6), 0)),
             (26.8, mk_q1("q1h0", wq1_col, range(6, 8), 0)),
             (27.6, mk_q1_evict("q1h0", 1, 0))]
            + [(32.0 + kt, mk_q1("q0h1", wq0_col, range(kt, kt + 1), 1))
               for kt in range(NK)]
            + [(40.0, mk_q1_evict("q0h1", 0, 1))]
            + [(41.0 + kt, mk_q1("q1h1", wq1_col, range(kt, kt + 1), 1))
               for kt in range(NK)]
            + [(49.0, mk_q1_evict("q1h1", 1, 1)), (49.5, t_close_p1b)]
        )

        outps_stack = ExitStack()
        outps = None
        pvk_stack = ExitStack()
        pq_stack = ExitStack()

        def mk_outproj(sc_i, n, p):
            # block-0 out-projection fill-in: one matmul per task (~213ns)
            # so it never blows the per-exp-slot PE budget
            def t():
                nonlocal outps
                if outps is None:
                    outps = outps_stack.enter_context(
                        tc.tile_pool(name="outps", bufs=2, space="PSUM", side="right"))
                s = sc_i
                half = sc_i % 2
                if half == 0 and n == 0 and p == 0:
                    state["osb"] = osbp.tile([128, 2 * D], BF16, name="osb")
                o_sb = state["osb"]
                if p == 0:
                    state["ops"] = outps.tile([128, W], F32, name="ops")
                o_ps = state["ops"]
                nc.tensor.matmul(
                    o_ps[:],
                    lhsT=ht_sb[:, p * S + s * 128:p * S + (s + 1) * 128],
                    rhs=wo_sb[:, p * D + n * W:p * D + (n + 1) * W],
                    start=(p == 0), stop=(p == 1),
                )
                if p == 1:
                    nc.vector.tensor_copy(
                        o_sb[:, half * D + n * W:half * D + (n + 1) * W], o_ps[:])
                if half == 1 and n == 1 and p == 1:
                    nc.sync.dma_start(
                        part[(s - 1) * 128:(s + 1) * 128, :].rearrange(
                            "(c p) d -> p c d", p=128),
                        o_sb[:].rearrange("p (c d) -> p c d", d=D),
                    )
                    if s == (HALF // 128) - 1:
                        outps_stack.close()
            return t

        hp = None
        slot_idx = 0
        for bI in range(2):
            for h in range(HG):
                pt, hi = h // 2, h % 2
                qbase = pt * S + bI * HALF
                if hi == 0:
                    hp = hpp.tile([128, HALF], BF16, name="hp")
                hu = [hups.tile([128, 260], F32, name=f"hu{g}") for g in range(2)]
                ets = [None] * NSK
                # head 0 needs the deep PV pipeline so the deferred V'
                # chains' writes are emitted before the first PV read;
                # later heads only need enough depth to cover the previous
                # head's norm-eviction WAR on the hu banks
                pv_depth = 10 if (bI == 0 and h == 0) else 6

                def emit_pv(sk, qts=range(8), hu=hu, ets=ets):
                    for qt in qts:
                        nc.tensor.matmul(
                            hu[qt // 4][:, (qt % 4) * 65:(qt % 4) * 65 + 65],
                            lhsT=ets[sk][:, qt * 128:(qt + 1) * 128],
                            rhs=v1_sb[:, sk * 65:(sk + 1) * 65],
                            # start zeroes the whole 2KB PSUM bank (all 4 qt
                            # regions), so only the bank's first/last matmul
                            # opens/closes the accumulation group
                            start=(sk == 0 and qt % 4 == 0),
                            stop=(sk == NSK - 1 and qt % 4 == 3),
                        )

                for sk in range(NSK):
                    # PV trails exp by 10 slots: the first PV of a head waits
                    # the previous head's norm-eviction (hu banks) and, in
                    # head 0, the deferred V' chains (whose writes must also
                    # be EMITTED before the first PV read to get the RAW
                    # dependency right); the deep pipeline keeps those waits
                    # off the PE queue's critical path.
                    if sk >= pv_depth:
                        emit_pv(sk - pv_depth)
                    sc = scps.tile([128, HALF], F32, name="sc")
                    for n in range(2):
                        nc.tensor.matmul(
                            sc[:, n * W:(n + 1) * W],
                            lhsT=kt2_sb[hi * 64:(hi + 1) * 64, sk * 128:(sk + 1) * 128],
                            rhs=qt_sb[hi * 64:(hi + 1) * 64, qbase + n * W:qbase + (n + 1) * W],
                            start=True, stop=True,
                        )
                    et = expp.tile([128, HALF], BF16, name="et")
                    nc.scalar.activation(et[:], sc[:], EXP, scale=0.125)
                    ets[sk] = et
                    if tasks:
                        floor_us, t = tasks.popleft()
                        with tc.tile_wait_until(floor_us / 1000.0):
                            t()
                    slot_idx += 1
                # drain each hu bank group separately so its norm-eviction
                # and hT transpose fire before the other group's PV tail
                rec = smalls.tile([128, 8], F32, name="rec")
                for g in range(2):
                    qts = range(g * 4, g * 4 + 4)
                    for sk in range(NSK - pv_depth, NSK):
                        emit_pv(sk, qts)
                    for q4 in range(4):
                        nc.vector.reciprocal(
                            rec[:, g * 4 + q4:g * 4 + q4 + 1],
                            hu[g][:, q4 * 65 + 64:q4 * 65 + 65],
                        )
                    for qt in qts:
                        dstn = hp[:, qt * 128 + hi * 64:qt * 128 + hi * 64 + 64]
                        srcn = hu[g][:, (qt % 4) * 65:(qt % 4) * 65 + 64]
                        if bI == 1 and h == HG - 1 and qt % 2 == 1:
                            # last head: ACT is done with exp, splitting the
                            # norm evictions halves the tail's norm latency
                            nc.scalar.activation(
                                dstn, srcn, mybir.ActivationFunctionType.Copy,
                                scale=rec[:, qt:qt + 1],
                            )
                        else:
                            nc.vector.tensor_scalar_mul(dstn, srcn,
                                                        rec[:, qt:qt + 1])
                    if hi == 1:
                        lo = g * 4 * 128
                        c0 = pt * S + bI * HALF + lo
                        if bI == 1 and pt == 1:
                            # final pair: PE transposes + eviction (~1us
                            # lower latency than the DMA-transpose path,
                            # directly shortening the kernel tail)
                            if "trtp" not in state:
                                state["trt_stack"] = ExitStack()
                                state["trtp"] = state["trt_stack"].enter_context(
                                    tc.tile_pool(name="trtp", bufs=2,
                                                 space="PSUM", side="right"))
                            trt = state["trtp"].tile([128, 4 * 128], BF16, name="trt")
                            for q4 in range(4):
                                nc.tensor.transpose(
                                    trt[:, q4 * 128:(q4 + 1) * 128],
                                    hp[:, lo + q4 * 128:lo + (q4 + 1) * 128],
                                    ident[:],
                                )
                            nc.vector.tensor_copy(ht_sb[:, c0:c0 + 4 * 128], trt[:])
                        else:
                            dst = ht_sb[:, c0:c0 + 4 * 128]
                            dst = dst.rearrange("p (a b) -> p a b", b=128)
                            nc.sync.dma_start_transpose(dst, hp[:, lo:lo + 4 * 128])
                if hi == 1 and pt == 1 and bI == 0:
                    j = 0
                    for sc_i in range(HALF // 128):
                        for n in range(2):
                            for p in range(2):
                                tasks.append((82.0 + j * 1.04,
                                              mk_outproj(sc_i, n, p)))
                                j += 1
        while tasks:
            floor_us, t = tasks.popleft()
            with tc.tile_wait_until(floor_us / 1000.0):
                t()
        if outps is not None:
            outps_stack.close()

        # ---- final flush: block-1 out-projection with the reclaimed
        # scores/h_un banks giving a 4-deep PSUM pipeline ----------------
        if "trt_stack" in state:
            state.pop("trt_stack").close()
            state.pop("trtp")
        scph_stack.close()
        with tc.tile_pool(name="outpsB", bufs=6, space="PSUM") as outpsB:
            for sc_i in range(HALF // 128):
                s = (HALF // 128) + sc_i
                half = sc_i % 2
                if half == 0:
                    state["osbB"] = osbp.tile([128, 2 * D], BF16, name="osbB")
                o_sb = state["osbB"]
                for n in range(2):
                    o_ps = outpsB.tile([128, W], F32, name="opsB")
                    for p in range(2):
                        nc.tensor.matmul(
                            o_ps[:],
                            lhsT=ht_sb[:, p * S + s * 128:p * S + (s + 1) * 128],
                            rhs=wo_sb[:, p * D + n * W:p * D + (n + 1) * W],
                            start=(p == 0), stop=(p == 1),
                        )
                    dst = o_sb[:, half * D + n * W:half * D + (n + 1) * W]
                    # 2:1 DVE:ACT eviction split keeps both engines under
                    # the ~854ns/chunk PE cadence
                    if n == 1 and sc_i % 3 != 2:
                        nc.scalar.copy(dst, o_ps[:])
                    else:
                        nc.vector.tensor_copy(dst, o_ps[:])
                if sc_i >= 6:
                    # last two chunks store singly: the final DMA then
                    # trails the last eviction by ~0.7us less
                    nc.sync.dma_start(
                        part[s * 128:(s + 1) * 128, :],
                        o_sb[:, half * D:(half + 1) * D],
                    )
                elif half == 1:
                    nc.sync.dma_start(
                        part[(s - 1) * 128:(s + 1) * 128, :].rearrange(
                            "(c p) d -> p c d", p=128),
                        o_sb[:].rearrange("p (c d) -> p c d", d=D),
                    )
        if _DEBUG:
            nc.sync.dma_start(dbg["d_qt"][:, :], qt_sb[:])
            nc.sync.dma_start(dbg["d_kt2"][:, :], kt2_sb[:])
            nc.sync.dma_start(dbg["d_v1"][:, :], v1_sb[:])
            nc.sync.dma_start(dbg["d_ht"][:, :], ht_sb[:])

    nc.finalize()
    return nc


def _get_nc():
    if "nc" not in _CACHE:
        _CACHE["nc"] = _build_nc()
    return _CACHE["nc"]


def _prep_core_inputs(inputs, wq, bq, wk, wv, wo):
    """Host-side shard prep: per-core transposed/rearranged bf16 operands."""
    from ml_dtypes import bfloat16

    xT = [np.ascontiguousarray(np.asarray(inputs[b], np.float32).T).astype(bfloat16)
          for b in range(B)]
    wq3 = np.asarray(wq, np.float32).reshape(Dh, NUM_HEADS, D)
    bq2 = np.asarray(bq, np.float32).reshape(Dh, NUM_HEADS)
    wkvT = np.ascontiguousarray(
        np.concatenate([np.asarray(wk, np.float32).T, np.asarray(wv, np.float32).T],
                       axis=1)
    ).astype(bfloat16)  # [1024, 128], K in cols 0:64
    wo_ = np.asarray(wo, np.float32)

    in_maps = []
    for c in range(N_CORES):
        b, g = divmod(c, G)
        heads = [g * HG + hl for hl in range(HG)]
        wqT_g = np.ascontiguousarray(
            np.concatenate([wq3[:, h, :].T for h in heads], axis=1)
        ).astype(bfloat16)
        bq_g = np.ascontiguousarray(
            np.concatenate([bq2[:, h] for h in heads]).reshape(QD, 1).astype(np.float32)
        )
        woT_g = np.ascontiguousarray(
            wo_[:, g * QD:(g + 1) * QD].T
        ).astype(bfloat16)  # [256, 1024]
        in_maps.append({
            "xT": xT[b],
            "wqT": wqT_g,
            "wkvT": wkvT,
            "woT": woT_g,
            "bq": bq_g,
        })
    return in_maps


def kernel(inputs, wq, bq, wk, bk, wv, bv, wo, bo):
    from concourse.bass_utils import run_bass_kernel_spmd

    nc = _get_nc()
    in_maps = _prep_core_inputs(inputs, wq, bq, wk, wv, wo)
    res = run_bass_kernel_spmd(nc, in_maps, list(range(N_CORES))).results

    wo_ = np.asarray(wo, np.float32)
    bias = (
        np.asarray(bo, np.float32)
        + wo_ @ np.tile(np.asarray(bv, np.float32), NUM_HEADS)
    )
    out = np.empty((B, S, D), np.float32)
    for b in range(B):
        acc = res[b * G]["part"].astype(np.float32).copy()
        for g in range(1, G):
            acc += res[b * G + g]["part"]
        out[b] = acc + bias
    return out


# revision 88
# speedup vs baseline: 1.0015x; 1.0015x over previous
"""MQA self-attention kernel for Trainium2, 8 NeuronCores.

Reference computation (fp32):
    q = x @ wq.T + bq        -> [B,S,1024] -> heads via (hidden num_heads) split
    k = x @ wk.T + bk        -> [B,S,64]  (single shared KV head)
    v = x @ wv.T + bv
    scores = q @ k.T / 8 ; attn = softmax(scores) ; h = attn @ v
    out = merge_heads(h) @ wo.T + bo

Sharding (8 cores, no collectives): core c handles batch b=c//4 and head
group g=c%4 (4 of the 16 q-heads).  The shared K/V head is replicated.
Each core returns the partial output h_g @ wo_g.T [S, D]; the host sums
the 4 head-group partials per batch and adds the bias terms.

Math notes:
 - bk provably cancels in softmax; bv is folded into the host-side output
   bias (softmax rows sum to 1); softmax runs without max subtraction
   (scores ~ N(0,1), exp stays within bf16/f32 range).

Device pipeline (all operands bf16, PSUM f32; the Activation engine's
exp throughput ~133us/core is the hard floor, so every other engine is
kept strictly below it):
 - xT/weights land as bf16 (halves the input DMA); issue queues are
   split SP/ACT because each DMA costs ~1.2-2us of serialized issue.
 - projections produce QT [2 heads stacked per 128 partitions] and KT
   (two partition-offset copies so odd/even heads get offset-matched
   operands); V' = [V|1] is computed DIRECTLY in [keys, dims] layout
   (lhsT = xT chunk, rhs = wvT) so no transposes or eviction chains
   sit in front of the early scores.
 - scores_T[k, q] per head per 1024-query block; exp on ScalarE in
   [128,1024] blocks writing bf16.
 - PV runs dense: h_un[q, 65] += exp_T[:, qtile].T @ V' accumulated over
   key tiles in PSUM (2x fewer PE rows than the hT-layout alternative).
   PSUM accumulation groups are per-2KB-bank: one start/stop per bank.
 - normalize on eviction via per-partition reciprocal multiply; pairs of
   heads share an SBUF buffer flipped to hT layout by one DMA-transpose
   per 512-col group (PE-transpose for the final pair, lower latency).
 - half-1 projections, V' chains and out-projection chunks are deferred
   PE tasks popped one per exp slot; emission order encodes the RAW/WAR
   dependencies while tile_wait_until schedule floors stop the greedy
   scheduler from hoisting them in front of the critical exp chain.
 - the final block's out-projection flushes after the scores/h_un banks
   are reclaimed, with a 4-deep PSUM pipeline, 2:1 DVE:ACT evictions
   and 2-chunk batched output DMAs.
"""

from collections import deque

import numpy as np

NUM_HEADS = 16
Dh = 64
B, S, D = 2, 2048, 1024
G = 4            # head groups (cores per batch)
HG = 4           # heads per group
QD = HG * Dh     # 256 local q dims
NK = D // 128    # 8 contraction tiles for projections
NSK = S // 128   # 16 key tiles
W = 512          # matmul moving width
HALF = 1024      # query block / projection column half
N_CORES = 8

_CACHE = {}
_DEBUG = False


def _build_nc():
    from contextlib import ExitStack

    import concourse.bass as bass
    import concourse.mybir as mybir
    import concourse.tile as tile
    from concourse import bacc
    from concourse.masks import make_identity

    F32 = mybir.dt.float32
    BF16 = mybir.dt.bfloat16
    EXP = mybir.ActivationFunctionType.Exp

    nc = bacc.Bacc("TRN2", target_bir_lowering=False, debug=False)

    xT = nc.declare_dram_parameter("xT", [D, S], BF16, isOutput=False)
    wqT = nc.declare_dram_parameter("wqT", [D, QD], BF16, isOutput=False)
    wkvT = nc.declare_dram_parameter("wkvT", [D, 128], BF16, isOutput=False)
    woT = nc.declare_dram_parameter("woT", [QD, D], BF16, isOutput=False)
    bqp = nc.declare_dram_parameter("bq", [QD, 1], F32, isOutput=False)
    part = nc.declare_dram_parameter("part", [S, D], BF16, isOutput=True)
    if _DEBUG:
        dbg = {
            "d_qt": nc.declare_dram_parameter("d_qt", [128, 2 * S], BF16, isOutput=True),
            "d_kt2": nc.declare_dram_parameter("d_kt2", [128, S], BF16, isOutput=True),
            "d_v1": nc.declare_dram_parameter("d_v1", [128, NSK * 65], BF16, isOutput=True),
            "d_ht": nc.declare_dram_parameter("d_ht", [128, 2 * S], BF16, isOutput=True),
        }

    with tile.TileContext(nc) as tc, ExitStack() as ctx:
        const = ctx.enter_context(tc.tile_pool(name="const", bufs=1))
        persist = ctx.enter_context(tc.tile_pool(name="persist", bufs=1))

        wq_sb = const.tile([128, NK * QD], BF16)    # ktile kt at cols [kt*QD:+QD]
        wkv_sb = const.tile([128, NK * 128], BF16)  # cols 0:64 = wkT, 64:128 = wvT
        wo_sb = const.tile([128, 2 * D], BF16)      # q-ktile p at cols [p*D:+D]
        bq_sb = const.tile([128, 2], F32)
        ident = const.tile([128, 128], BF16)        # for the tail PE transposes

        qt_sb = persist.tile([128, 2 * S], BF16)    # pt p cols [p*S:+S]; rows 0:64 head 2p, 64:128 head 2p+1
        kt2_sb = persist.tile([128, S], BF16)       # KT duplicated rows 0:64 and 64:128
        v1_sb = persist.tile([128, NSK * 65], BF16)  # V' tile sk at cols [sk*65:+65]
        ht_sb = persist.tile([128, 2 * S], BF16)    # hT, q-ktile p at cols [p*S:+S]

        make_identity(nc, ident[:])
        nc.vector.memset(v1_sb[:], 1.0)  # pre-fill the softmax-denominator columns

        # ---- DMAs + half-0 projections --------------------------------
        # x lands in 4 tiles of 2 ktiles (half 0, fine overlap with the
        # projection chains) + 2 tiles of 4 ktiles (half 1).  Issue queues
        # are spread over SP/ACT/DVE so the ~1.2us per-DMA issue cost
        # doesn't serialize the critical path.
        xp = ctx.enter_context(tc.tile_pool(name="xp", bufs=1))
        x0 = [xp.tile([128, 2 * HALF], BF16, name=f"x0_{j}") for j in range(4)]
        x1 = [xp.tile([128, 2 * HALF], BF16, name=f"x1_{j}") for j in range(4)]

        def dma_x(eng, tiles, j, hf):
            eng.dma_start(
                tiles[j][:].rearrange("p (k c) -> p k c", c=HALF),
                xT[j * 256:(j + 1) * 256, hf * HALF:(hf + 1) * HALF].rearrange(
                    "(k p) c -> p k c", p=128),
            )

        # weights issue from the ACT queue (idle until the first exp at
        # ~12us), x tiles from SP: neither queue's serialized ~1.2-2us
        # per-DMA issue cost then gates the critical path
        nc.scalar.dma_start(
            wkv_sb[:].rearrange("p (k c) -> p k c", c=128),
            wkvT[:, :].rearrange("(k p) c -> p k c", p=128),
        )
        nc.scalar.dma_start(
            wq_sb[:].rearrange("p (k c) -> p k c", c=QD),
            wqT[:, :].rearrange("(k p) c -> p k c", p=128),
        )
        for j in range(4):
            dma_x(nc.sync, x0, j, 0)
        for j in range(4):
            dma_x(nc.sync, x1, j, 1)
        for p in range(2):
            nc.scalar.dma_start(bq_sb[:, p:p + 1], bqp[p * 128:(p + 1) * 128, :])
        nc.scalar.dma_start(
            wo_sb[:].rearrange("p (a c) -> p a c", c=D),
            woT[:, :].rearrange("(a p) c -> p a c", p=128),
        )

        def xcol(kt, hf, n):
            tiles = x0 if hf == 0 else x1
            return tiles[kt // 2][:, (kt % 2) * HALF + n * W:(kt % 2) * HALF + (n + 1) * W]

        def proj_kt_step(ps, wsb_col, kt, hf, width):
            for n in range(HALF // W):
                nc.tensor.matmul(
                    ps[:, n * W:(n + 1) * W],
                    lhsT=wsb_col(kt),
                    rhs=xcol(kt, hf, n),
                    start=(kt == 0), stop=(kt == NK - 1),
                )

        wkv_col = lambda kt: wkv_sb[:, kt * 128:(kt + 1) * 128]
        wq0_col = lambda kt: wq_sb[:, kt * QD:kt * QD + 128]
        wq1_col = lambda kt: wq_sb[:, kt * QD + 128:kt * QD + 256]

        with tc.tile_pool(name="p0", bufs=1, space="PSUM") as p0:
            vk_ps = p0.tile([128, HALF], F32, name="vk")
            q0_ps = p0.tile([128, HALF], F32, name="q0")
            # vk + q0 gate the first exp; everything else is deferred with
            # schedule-time floors so it can't precede the early scores
            for kt in range(NK):
                proj_kt_step(vk_ps, wkv_col, kt, 0, HALF)
                proj_kt_step(q0_ps, wq0_col, kt, 0, HALF)
            # halved evictions: the first scores matmul only needs the first
            # key tile and query half, so don't gate it on full-width copies
            nc.scalar.copy(kt2_sb[0:64, 0:W], vk_ps[0:64, 0:W])
            nc.vector.tensor_scalar_add(qt_sb[:, 0:W], q0_ps[:, 0:W], bq_sb[:, 0:1])
            nc.scalar.copy(kt2_sb[0:64, W:HALF], vk_ps[0:64, W:HALF])
            nc.vector.tensor_scalar_add(qt_sb[:, W:HALF], q0_ps[:, W:HALF],
                                        bq_sb[:, 0:1])
            nc.gpsimd.tensor_copy(kt2_sb[64:128, 0:HALF], kt2_sb[0:64, 0:HALF])

        # ---- Phase 2: attention, with deferred fill-in PE tasks -------
        expp = ctx.enter_context(tc.tile_pool(name="expp", bufs=14))
        hpp = ctx.enter_context(tc.tile_pool(name="hpp", bufs=2))
        smalls = ctx.enter_context(tc.tile_pool(name="smalls", bufs=4))
        osbp = ctx.enter_context(tc.tile_pool(name="osbp", bufs=3))
        # scores/h_un PSUM lives in its own stack so the final-block flush
        # can reclaim those banks for a deeper out-projection pipeline.
        # hups is created first: its banks land on the just-freed phase-1
        # chain banks (whose WAR hold clears before the first PV), keeping
        # the second scores buffer's banks free of phase-1 dependencies.
        scph_stack = ExitStack()
        hups = scph_stack.enter_context(tc.tile_pool(name="hups", bufs=1, space="PSUM"))
        scps = scph_stack.enter_context(tc.tile_pool(name="scps", bufs=2, space="PSUM"))

        state = {}

        def mk_vk1(kts):
            def t():
                if "vk1" not in state:
                    pvk = pvk_stack.enter_context(
                        tc.tile_pool(name="pvk", bufs=1, space="PSUM", side="right"))
                    state["vk1"] = pvk.tile([128, HALF], F32, name="vkps")
                for kt in kts:
                    proj_kt_step(state["vk1"], wkv_col, kt, 1, HALF)
            return t

        def t_kv1_evict_a():
            # first half of the K eviction unblocks scores sk8-11 sooner
            nc.vector.tensor_copy(kt2_sb[0:64, HALF:HALF + W],
                                  state["vk1"][0:64, 0:W])

        def t_kv1_evict_b():
            vk1 = state.pop("vk1")
            nc.vector.tensor_copy(kt2_sb[0:64, HALF + W:S], vk1[0:64, W:HALF])
            nc.gpsimd.tensor_copy(kt2_sb[64:128, HALF:S], kt2_sb[0:64, HALF:S])
            pvk_stack.close()

        def mk_vdirect(era, sks, close=False):
            # V in [keys, dims] layout computed directly (lhsT = xT chunk,
            # rhs = wvT): no PE transposes, no eviction dependency chains —
            # each key tile is its own 8-matmul chain plus one DVE eviction
            def t():
                if era not in state:
                    state[era] = ExitStack()
                    state[era + "_p"] = state[era].enter_context(
                        tc.tile_pool(name=era, bufs=2, space="PSUM", side="right"))
                for sk in sks:
                    vd = state[era + "_p"].tile([128, Dh], F32, name="vd")
                    hf, c0 = (0, sk * 128) if sk < NSK // 2 else (1, sk * 128 - HALF)
                    tiles = x0 if hf == 0 else x1
                    for kt in range(NK):
                        nc.tensor.matmul(
                            vd[:],
                            lhsT=tiles[kt // 2][:, (kt % 2) * HALF + c0:
                                                (kt % 2) * HALF + c0 + 128],
                            rhs=wkv_sb[:, kt * 128 + 64:kt * 128 + 128],
                            start=(kt == 0), stop=(kt == NK - 1),
                        )
                    nc.vector.tensor_copy(v1_sb[:, sk * 65:sk * 65 + 64], vd[:])
                if close:
                    state.pop(era).close()
                    state.pop(era + "_p")
            return t

        def mk_q1(which, col_fn, kts, hf):
            def t():
                if which in state:
                    ps = state[which]
                else:
                    if "pq" not in state:
                        state["pq"] = pq_stack.enter_context(
                            tc.tile_pool(name="pq", bufs=1, space="PSUM", side="right"))
                    ps = state[which] = state["pq"].tile([128, HALF], F32, name="qps")
                for kt in kts:
                    proj_kt_step(ps, col_fn, kt, hf, HALF)
            return t

        def mk_q1_evict(which, pt, hf):
            def t():
                ps = state.pop(which)
                c0 = pt * S + hf * HALF
                nc.vector.tensor_scalar_add(
                    qt_sb[:, c0:c0 + HALF], ps[:], bq_sb[:, pt:pt + 1]
                )
            return t

        def t_close_p1b():
            pq_stack.close()

        # (floor_us, task): the floor is a schedule-sim wait that keeps this
        # fill-in work from being greedily ordered ahead of the exp chain
        tasks = deque(
            [(12.6, mk_vk1(range(0, 2))), (14.1, mk_vk1(range(2, 4))),
             (15.6, mk_vk1(range(4, 6))), (17.1, mk_vk1(range(6, 8))),
             (17.7, t_kv1_evict_a), (18.1, t_kv1_evict_b),
             (17.2, mk_vdirect("vd", (0, 1))),
             (18.0, mk_vdirect("vd", (2, 3))),
             (18.8, mk_vdirect("vd", (4, 5))),
             (19.6, mk_vdirect("vd", (6, 7))),
             (20.4, mk_vdirect("vd", (8, 9))),
             (21.2, mk_vdirect("vd", (10, 11))),
             (22.0, mk_vdirect("vd", (12, 13))),
             (22.8, mk_vdirect("vd", (14, 15), close=True)),
             (24.4, mk_q1("q1h0", wq1_col, range(0, 2), 0)),
             (25.2, mk_q1("q1h0", wq1_col, range(2, 4), 0)),
             (26.0, mk_q1("q1h0", wq1_col, range(4, 6), 0)),
             (26.8, mk_q1("q1h0", wq1_col, range(6, 8), 0)),
             (27.6, mk_q1_evict("q1h0", 1, 0))]
            + [(32.0 + kt, mk_q1("q0h1", wq0_col, range(kt, kt + 1), 1))
               for kt in range(NK)]
            + [(40.0, mk_q1_evict("q0h1", 0, 1))]
            + [(41.0 + kt, mk_q1("q1h1", wq1_col, range(kt, kt + 1), 1))
               for kt in range(NK)]
            + [(49.0, mk_q1_evict("q1h1", 1, 1)), (49.5, t_close_p1b)]
        )

        outps_stack = ExitStack()
        outps = None
        pvk_stack = ExitStack()
        pq_stack = ExitStack()

        def mk_outproj(sc_i, n, p):
            # block-0 out-projection fill-in: one matmul per task (~213ns)
            # so it never blows the per-exp-slot PE budget
            def t():
                nonlocal outps
                if outps is None:
                    outps = outps_stack.enter_context(
                        tc.tile_pool(name="outps", bufs=2, space="PSUM", side="right"))
                s = sc_i
                half = sc_i % 2
                if half == 0 and n == 0 and p == 0:
                    state["osb"] = osbp.tile([128, 2 * D], BF16, name="osb")
                o_sb = state["osb"]
                if p == 0:
                    state["ops"] = outps.tile([128, W], F32, name="ops")
                o_ps = state["ops"]
                nc.tensor.matmul(
                    o_ps[:],
                    lhsT=ht_sb[:, p * S + s * 128:p * S + (s + 1) * 128],
                    rhs=wo_sb[:, p * D + n * W:p * D + (n + 1) * W],
                    start=(p == 0), stop=(p == 1),
                )
                if p == 1:
                    nc.vector.tensor_copy(
                        o_sb[:, half * D + n * W:half * D + (n + 1) * W], o_ps[:])
                if half == 1 and n == 1 and p == 1:
                    nc.sync.dma_start(
                        part[(s - 1) * 128:(s + 1) * 128, :].rearrange(
                            "(c p) d -> p c d", p=128),
                        o_sb[:].rearrange("p (c d) -> p c d", d=D),
                    )
                    if s == (HALF // 128) - 1:
                        outps_stack.close()
            return t

        hp = None
        slot_idx = 0
        for bI in range(2):
            for h in range(HG):
                pt, hi = h // 2, h % 2
                qbase = pt * S + bI * HALF
                if hi == 0:
                    hp = hpp.tile([128, HALF], BF16, name="hp")
                hu = [hups.tile([128, 260], F32, name=f"hu{g}") for g in range(2)]
                ets = [None] * NSK
                # head 0 needs the deep PV pipeline so the deferred V'
                # chains' writes are emitted before the first PV read;
                # later heads only need enough depth to cover the previous
                # head's norm-eviction WAR on the hu banks
                pv_depth = 10 if (bI == 0 and h == 0) else 6

                def emit_pv(sk, qts=range(8), hu=hu, ets=ets):
                    for qt in qts:
                        nc.tensor.matmul(
                            hu[qt // 4][:, (qt % 4) * 65:(qt % 4) * 65 + 65],
                            lhsT=ets[sk][:, qt * 128:(qt + 1) * 128],
                            rhs=v1_sb[:, sk * 65:(sk + 1) * 65],
                            # start zeroes the whole 2KB PSUM bank (all 4 qt
                            # regions), so only the bank's first/last matmul
                            # opens/closes the accumulation group
                            start=(sk == 0 and qt % 4 == 0),
                            stop=(sk == NSK - 1 and qt % 4 == 3),
                        )

                for sk in range(NSK):
                    # PV trails exp by 10 slots: the first PV of a head waits
                    # the previous head's norm-eviction (hu banks) and, in
                    # head 0, the deferred V' chains (whose writes must also
                    # be EMITTED before the first PV read to get the RAW
                    # dependency right); the deep pipeline keeps those waits
                    # off the PE queue's critical path.
                    if sk >= pv_depth:
                        emit_pv(sk - pv_depth)
                    sc = scps.tile([128, HALF], F32, name="sc")
                    for n in range(2):
                        nc.tensor.matmul(
                            sc[:, n * W:(n + 1) * W],
                            lhsT=kt2_sb[hi * 64:(hi + 1) * 64, sk * 128:(sk + 1) * 128],
                            rhs=qt_sb[hi * 64:(hi + 1) * 64, qbase + n * W:qbase + (n + 1) * W],
                            start=True, stop=True,
                        )
                    et = expp.tile([128, HALF], BF16, name="et")
                    nc.scalar.activation(et[:], sc[:], EXP, scale=0.125)
                    ets[sk] = et
                    if tasks:
                        floor_us, t = tasks.popleft()
                        with tc.tile_wait_until(floor_us / 1000.0):
                            t()
                    slot_idx += 1
                # drain each hu bank group separately so its norm-eviction
                # and hT transpose fire before the other group's PV tail
                rec = smalls.tile([128, 8], F32, name="rec")
                for g in range(2):
                    qts = range(g * 4, g * 4 + 4)
                    for sk in range(NSK - pv_depth, NSK):
                        emit_pv(sk, qts)
                    for q4 in range(4):
                        nc.vector.reciprocal(
                            rec[:, g * 4 + q4:g * 4 + q4 + 1],
                            hu[g][:, q4 * 65 + 64:q4 * 65 + 65],
                        )
                    for qt in qts:
                        dstn = hp[:, qt * 128 + hi * 64:qt * 128 + hi * 64 + 64]
                        srcn = hu[g][:, (qt % 4) * 65:(qt % 4) * 65 + 64]
                        if bI == 1 and h == HG - 1 and qt % 2 == 1:
                            # last head: ACT is done with exp, splitting the
                            # norm evictions halves the tail's norm latency
                            nc.scalar.activation(
                                dstn, srcn, mybir.ActivationFunctionType.Copy,
                                scale=rec[:, qt:qt + 1],
                            )
                        else:
                            nc.vector.tensor_scalar_mul(dstn, srcn,
                                                        rec[:, qt:qt + 1])
                    if hi == 1:
                        lo = g * 4 * 128
                        c0 = pt * S + bI * HALF + lo
                        if bI == 1 and pt == 1:
                            # final pair: PE transposes + eviction (~1us
                            # lower latency than the DMA-transpose path,
                            # directly shortening the kernel tail)
                            if "trtp" not in state:
                                state["trt_stack"] = ExitStack()
                                state["trtp"] = state["trt_stack"].enter_context(
                                    tc.tile_pool(name="trtp", bufs=2,
                                                 space="PSUM", side="right"))
                            trt = state["trtp"].tile([128, 4 * 128], BF16, name="trt")
                            for q4 in range(4):
                                nc.tensor.transpose(
                                    trt[:, q4 * 128:(q4 + 1) * 128],
                                    hp[:, lo + q4 * 128:lo + (q4 + 1) * 128],
                                    ident[:],
                                )
                            nc.vector.tensor_copy(ht_sb[:, c0:c0 + 4 * 128], trt[:])
                        else:
                            dst = ht_sb[:, c0:c0 + 4 * 128]
                            dst = dst.rearrange("p (a b) -> p a b", b=128)
                            nc.sync.dma_start_transpose(dst, hp[:, lo:lo + 4 * 128])
                if hi == 1 and pt == 1 and bI == 0:
                    j = 0
                    for sc_i in range(HALF // 128):
                        for n in range(2):
                            for p in range(2):
                                tasks.append((82.0 + j * 1.04,
                                              mk_outproj(sc_i, n, p)))
                                j += 1
        while tasks:
            floor_us, t = tasks.popleft()
            with tc.tile_wait_until(floor_us / 1000.0):
                t()
        if outps is not None:
            outps_stack.close()

        # ---- final flush: block-1 out-projection with the reclaimed
        # scores/h_un banks giving a 4-deep PSUM pipeline ----------------
        if "trt_stack" in state:
            state.pop("trt_stack").close()
            state.pop("trtp")
        scph_stack.close()
        with tc.tile_pool(name="outpsB", bufs=6, space="PSUM") as outpsB:
            for sc_i in range(HALF // 128):
                s = (HALF // 128) + sc_i
                half = sc_i % 2
                if half == 0:
                    state["osbB"] = osbp.tile([128, 2 * D], BF16, name="osbB")
                o_sb = state["osbB"]
                for n in range(2):
                    o_ps = outpsB.tile([128, W], F32, name="opsB")
                    for p in range(2):
                        nc.tensor.matmul(
                            o_ps[:],
                            lhsT=ht_sb[:, p * S + s * 128:p * S + (s + 1) * 128],
                            rhs=wo_sb[:, p * D + n * W:p * D + (n + 1) * W],
                            start=(p == 0), stop=(p == 1),
                        )
                    dst = o_sb[:, half * D + n * W:half * D + (n + 1) * W]
                    # 2:1 DVE:ACT eviction split keeps both engines under
                    # the ~854ns/chunk PE cadence
                    if n == 1 and sc_i % 3 != 2:
                        nc.scalar.copy(dst, o_ps[:])
                    else:
                        nc.vector.tensor_copy(dst, o_ps[:])
                if half == 1:
                    nc.sync.dma_start(
                        part[(s - 1) * 128:(s + 1) * 128, :].rearrange(
                            "(c p) d -> p c d", p=128),
                        o_sb[:].rearrange("p (c d) -> p c d", d=D),
                    )
        if _DEBUG:
            nc.sync.dma_start(dbg["d_qt"][:, :], qt_sb[:])
            nc.sync.dma_start(dbg["d_kt2"][:, :], kt2_sb[:])
            nc.sync.dma_start(dbg["d_v1"][:, :], v1_sb[:])
            nc.sync.dma_start(dbg["d_ht"][:, :], ht_sb[:])

    nc.finalize()
    return nc


def _get_nc():
    if "nc" not in _CACHE:
        _CACHE["nc"] = _build_nc()
    return _CACHE["nc"]


def _prep_core_inputs(inputs, wq, bq, wk, wv, wo):
    """Host-side shard prep: per-core transposed/rearranged bf16 operands."""
    from ml_dtypes import bfloat16

    xT = [np.ascontiguousarray(np.asarray(inputs[b], np.float32).T).astype(bfloat16)
          for b in range(B)]
    wq3 = np.asarray(wq, np.float32).reshape(Dh, NUM_HEADS, D)
    bq2 = np.asarray(bq, np.float32).reshape(Dh, NUM_HEADS)
    wkvT = np.ascontiguousarray(
        np.concatenate([np.asarray(wk, np.float32).T, np.asarray(wv, np.float32).T],
                       axis=1)
    ).astype(bfloat16)  # [1024, 128], K in cols 0:64
    wo_ = np.asarray(wo, np.float32)

    in_maps = []
    for c in range(N_CORES):
        b, g = divmod(c, G)
        heads = [g * HG + hl for hl in range(HG)]
        wqT_g = np.ascontiguousarray(
            np.concatenate([wq3[:, h, :].T for h in heads], axis=1)
        ).astype(bfloat16)
        bq_g = np.ascontiguousarray(
            np.concatenate([bq2[:, h] for h in heads]).reshape(QD, 1).astype(np.float32)
        )
        woT_g = np.ascontiguousarray(
            wo_[:, g * QD:(g + 1) * QD].T
        ).astype(bfloat16)  # [256, 1024]
        in_maps.append({
            "xT": xT[b],
            "wqT": wqT_g,
            "wkvT": wkvT,
            "woT": woT_g,
            "bq": bq_g,
        })
    return in_maps


def kernel(inputs, wq, bq, wk, bk, wv, bv, wo, bo):
    from concourse.bass_utils import run_bass_kernel_spmd

    nc = _get_nc()
    in_maps = _prep_core_inputs(inputs, wq, bq, wk, wv, wo)
    res = run_bass_kernel_spmd(nc, in_maps, list(range(N_CORES))).results

    wo_ = np.asarray(wo, np.float32)
    bias = (
        np.asarray(bo, np.float32)
        + wo_ @ np.tile(np.asarray(bv, np.float32), NUM_HEADS)
    )
    out = np.empty((B, S, D), np.float32)
    for b in range(B):
        acc = res[b * G]["part"].astype(np.float32).copy()
        for g in range(1, G):
            acc += res[b * G + g]["part"]
        out[b] = acc + bias
    return out
